# revision 1
# baseline (speedup 1.0000x reference)
"""MinLSTM fused kernel for TRN2 (8 NeuronCores, batch-parallel).

Math (verified equivalent to the reference's log-space form):
    zf = x@Wf+bf ; zi = x@Wi+bi ; zh = x@Wh+bh
    sf = sigmoid(zf) ; si = sigmoid(zi) ; sh = sigmoid(zh)
    g  = max(zh + 0.5, sh)
    p  = si / sf
    S  = 0.5 + cumsum(p * g, axis=time)
    out[:, 0, :]  = 0.5
    out[:, t+1, :] = S[t] / (1 + p[t])

Per core (one batch element): z^T [1536, 4096] via fp32r matmuls
(stationary = W chunk, moving = x^T chunk), elementwise in
[H-partition, T-free] orientation (per-partition bias APs, DVE
tensor_tensor_scan along T), writes out^T [512, 4096]; host transposes
back and prepends the t=0 column.

Engine split: PE = GEMM; ACT = 3 sigmoids (from PSUM, AP bias) + 2
reciprocals, table-set batched per h-chunk (sigmoid set, then recip set);
DVE = g-STT, p-mul, scan; GpSimd = u-mul, out-mul; DMA via HWDGE (sync).
"""
import numpy as np

_CACHE = {}

B, T, D, H = 8, 4096, 512, 512
H3 = 3 * H
NCORES = 8
N_HC = H // 128       # 4 h-chunks
N_D = D // 128        # 4 contraction chunks
N_PAIR = 4            # 4 psum groups per h-chunk, each [128, 1024] of T
PW = 1024             # psum tile width (2 banks)
HALF = 2048           # recip/scan batch width


def _install_tilefix():
    """This walrus build accepts only ONE sync wait per hardware instruction;
    Tile can emit several. Spill extras onto injected single-wait drains."""
    import concourse.tile as tile
    from concourse import mybir
    from concourse.vector_clock import ScopedClock

    if getattr(tile.TileContext, "_minlstm_patched", False):
        return
    orig_lower = tile.TileContext._lower_ordered_insts

    def _spill_waits(self, ordered):
        nc = self.nc
        for bb_name, insts in ordered.items():
            out = []
            for inst in insts:
                si = inst.sync_info
                if si is not None and len(si.on_wait) > 1 and inst.engine is not None:
                    waits = list(si.on_wait)
                    for w in waits[:-1]:
                        d = mybir.InstDrain(
                            name=nc.get_next_instruction_name(),
                            ins=[], outs=[], bass_is_fusable=False,
                            sync_info=mybir.SyncInfo(on_wait=[w], on_update=[]),
                        )
                        d.engine = inst.engine
                        out.append(d)
                    si.on_wait = [waits[-1]]
                out.append(inst)
            insts[:] = out
        return ordered

    def _patched_lower(self, ordered):
        return orig_lower(self, _spill_waits(self, ordered))

    def _split_drain_and_barrier(self, tick_clock, wait_clock):
        drain_inst = self.nc.sync.drain()
        wait_clock.add_sem_waits(
            drain_inst.ins, ScopedClock({None: tick_clock.global_clock})
        )
        si = drain_inst.ins.sync_info
        if si is not None and len(si.on_wait) > 1:
            waits = list(si.on_wait)
            si.on_wait = [waits[0]]
            for w in waits[1:]:
                extra = self.nc.sync.drain()
                esi = extra.ins.sync_info
                if esi is None:
                    extra.ins.sync_info = mybir.SyncInfo(on_wait=[w], on_update=[])
                else:
                    esi.on_wait = [w]
        self.nc.all_engine_barrier()
        assert self.sems is not None
        popped = self.nc._tile_sem_poison_stack.pop()
        assert popped is self._sem_poison
        self.nc.clear_and_free_semaphores(list(self.sems.allocated().values()))
        self.nc.all_engine_barrier()

    tile.TileContext._lower_ordered_insts = _patched_lower
    tile.TileContext._drain_and_barrier = _split_drain_and_barrier
    tile.TileContext._minlstm_patched = True


def _build():
    import concourse.bass as bass
    import concourse.tile as tile
    from concourse import mybir
    from concourse.tile_rust import add_dep_helper

    _install_tilefix()

    f32 = mybir.dt.float32
    f32r = mybir.dt.float32r
    AF = mybir.ActivationFunctionType
    ALU = mybir.AluOpType

    XW_COLS = T + H3  # 5632

    nc = bass.Bass("TRN2", target_bir_lowering=False, debug=False,
                   num_devices=NCORES)

    xw_d = nc.dram_tensor("xw", [D, XW_COLS], f32r, kind="ExternalInput").ap()
    bias_d = nc.dram_tensor("biases", [128, 16], f32, kind="ExternalInput").ap()
    out_d = nc.dram_tensor("out", [H, T], f32, kind="ExternalOutput").ap()

    prev_act = [None]

    def act_raw(eng, out, in_, func, bias=0.0, scale=1.0):
        inputs = [eng.lower_ap(in_)]
        for arg in (bias, scale, 0.0):
            if isinstance(arg, bass.AP):
                inputs.append(eng.lower_ap(arg))
            else:
                inputs.append(
                    mybir.ImmediateValue(dtype=f32, value=float(arg))
                )
        i = eng.add_instruction(
            mybir.InstActivation(
                name=eng.bass.get_next_instruction_name(),
                func=func, ins=inputs, outs=[eng.lower_ap(out)],
            )
        )
        if prev_act[0] is not None:
            add_dep_helper(i.ins, prev_act[0].ins, sync=False,
                           reason="ACT table-set order")
        prev_act[0] = i
        return i

    with tile.TileContext(nc) as tc:
        with (
            tc.tile_pool(name="xwp", bufs=1) as xwp,
            tc.tile_pool(name="cons", bufs=1) as cons,
            tc.tile_pool(name="ps", bufs=4, space="PSUM") as ps,
            tc.tile_pool(name="bigs", bufs=1) as bigs,
            tc.tile_pool(name="Spool", bufs=2) as Spool,
            tc.tile_pool(name="rpool", bufs=2) as rpool,
            tc.tile_pool(name="roll", bufs=2) as roll,
        ):
            xw = [
                xwp.tile([128, XW_COLS], f32r, tag=f"xw{d}", name=f"xw{d}")
                for d in range(N_D)
            ]
            bounds = [0, H3 + 1024, H3 + 2048, H3 + 3072, XW_COLS]
            for i in range(len(bounds) - 1):
                for d in range(N_D):
                    nc.sync.dma_start(
                        xw[d][:, bounds[i] : bounds[i + 1]],
                        xw_d[128 * d : 128 * (d + 1), bounds[i] : bounds[i + 1]],
                    )
            bt = cons.tile([128, 16], f32, tag="bt")
            nc.sync.dma_start(bt[:], bias_d[:])
            zero1 = cons.tile([128, 8], f32, tag="zero1")
            nc.vector.memset(zero1[:], 0.0)
            zero_bc = zero1[:, 0:1].broadcast_to([128, HALF])

            for h in range(N_HC):
                bf_ap = bt[:, h : h + 1]
                bi_ap = bt[:, 4 + h : 5 + h]
                bg_ap = bt[:, 8 + h : 9 + h]    # bh + 0.5
                bh_ap = bt[:, 12 + h : 13 + h]  # bh

                sf = bigs.tile([128, T], f32, tag="sf", name=f"sf{h}")
                si = bigs.tile([128, T], f32, tag="si", name=f"si{h}")
                g = bigs.tile([128, T], f32, tag="g", name=f"g{h}")

                # ---- GEMM + sigmoid-set phase ----
                wf = [xw[d][:, 128 * h : 128 * (h + 1)] for d in range(N_D)]
                wi = [xw[d][:, H + 128 * h : H + 128 * (h + 1)]
                      for d in range(N_D)]
                wh = [xw[d][:, 2 * H + 128 * h : 2 * H + 128 * (h + 1)]
                      for d in range(N_D)]
                for pr in range(N_PAIR):
                    zf_ps = ps.tile([128, PW], f32, tag="z", name=f"zf{h}_{pr}")
                    zi_ps = ps.tile([128, PW], f32, tag="z", name=f"zi{h}_{pr}")
                    zh_ps = ps.tile([128, PW], f32, tag="z", name=f"zh{h}_{pr}")
                    for half in range(2):
                        t0 = H3 + PW * pr + 512 * half
                        sl = slice(512 * half, 512 * (half + 1))
                        for d in range(N_D):
                            rhs = xw[d][:, t0 : t0 + 512]
                            st, sp = (d == 0), (d == N_D - 1)
                            nc.tensor.matmul(zf_ps[:, sl], wf[d], rhs,
                                             start=st, stop=sp)
                            nc.tensor.matmul(zi_ps[:, sl], wi[d], rhs,
                                             start=st, stop=sp)
                            nc.tensor.matmul(zh_ps[:, sl], wh[d], rhs,
                                             start=st, stop=sp)
                    csl = slice(PW * pr, PW * (pr + 1))
                    act_raw(nc.scalar, sf[:, csl], zf_ps[:], AF.Sigmoid,
                            bias=bf_ap)
                    act_raw(nc.scalar, si[:, csl], zi_ps[:], AF.Sigmoid,
                            bias=bi_ap)
                    sh = roll.tile([128, PW], f32, tag="sh")
                    act_raw(nc.scalar, sh[:], zh_ps[:], AF.Sigmoid, bias=bh_ap)
                    nc.vector.scalar_tensor_tensor(
                        out=g[:, csl], in0=zh_ps[:], scalar=bg_ap, in1=sh[:],
                        op0=ALU.add, op1=ALU.max,
                    )

                # ---- recip-set phase: R1A,R1B then R2A,R2B; p on DVE ----
                r1s, ps2, r2s = [], [], []
                for half in range(2):
                    sl = slice(HALF * half, HALF * (half + 1))
                    r1 = rpool.tile([128, HALF], f32, tag="r",
                                    name=f"r1_{h}_{half}")
                    act_raw(nc.scalar, r1[:], sf[:, sl], AF.Reciprocal)
                    p = bigs.tile([128, HALF], f32, tag="p",
                                  name=f"p{h}_{half}")
                    nc.vector.tensor_mul(p[:], si[:, sl], r1[:])
                    r1s.append(r1); ps2.append(p)
                for half in range(2):
                    r2 = rpool.tile([128, HALF], f32, tag="r",
                                    name=f"r2_{h}_{half}")
                    act_raw(nc.scalar, r2[:], ps2[half][:], AF.Reciprocal,
                            bias=1.0)
                    r2s.append(r2)

                # ---- scan + output ----
                S_prev = None
                for half in range(2):
                    sl = slice(HALF * half, HALF * (half + 1))
                    u = bigs.tile([128, HALF], f32, tag="u",
                                  name=f"u{h}_{half}")
                    nc.gpsimd.tensor_mul(u[:], ps2[half][:], g[:, sl])
                    S = Spool.tile([128, HALF], f32, tag="S",
                                   name=f"S{h}_{half}")
                    init = 0.5 if half == 0 else S_prev[:, HALF - 1 : HALF]
                    nc.vector.tensor_tensor_scan(
                        S[:], zero_bc, u[:], init, ALU.add, ALU.add
                    )
                    S_prev = S
                    o = bigs.tile([128, HALF], f32, tag="o",
                                  name=f"o{h}_{half}")
                    # split the post-scan tail: GP and DVE in parallel, DMA
                    # each 1024-piece as soon as it is ready
                    q = HALF // 2
                    nc.gpsimd.tensor_mul(o[:, 0:q], S[:, 0:q],
                                         r2s[half][:, 0:q])
                    nc.sync.dma_start(
                        out_d[128 * h : 128 * (h + 1),
                              HALF * half : HALF * half + q],
                        o[:, 0:q],
                    )
                    nc.vector.tensor_mul(o[:, q:HALF], S[:, q:HALF],
                                         r2s[half][:, q:HALF])
                    nc.sync.dma_start(
                        out_d[128 * h : 128 * (h + 1),
                              HALF * half + q : HALF * (half + 1)],
                        o[:, q:HALF],
                    )
    return nc


def _get_nc():
    if "nc" not in _CACHE:
        _CACHE["nc"] = _build()
    return _CACHE["nc"]


def _make_in_maps(x, Wf, bf, Wi, bi, Wh, bh):
    x = np.ascontiguousarray(np.asarray(x, dtype=np.float32))
    W_all = np.concatenate(
        [np.asarray(Wf), np.asarray(Wi), np.asarray(Wh)], axis=1
    ).astype(np.float32)

    biases = np.zeros((128, 16), dtype=np.float32)
    biases[:, 0:4] = np.asarray(bf, dtype=np.float32).reshape(N_HC, 128).T
    biases[:, 4:8] = np.asarray(bi, dtype=np.float32).reshape(N_HC, 128).T
    bh32 = np.asarray(bh, dtype=np.float32)
    biases[:, 8:12] = (bh32 + np.float32(0.5)).reshape(N_HC, 128).T
    biases[:, 12:16] = bh32.reshape(N_HC, 128).T

    in_maps = []
    for c in range(NCORES):
        xT = np.ascontiguousarray(x[c].T)
        xw = np.concatenate([W_all, xT], axis=1)
        in_maps.append({"xw": xw, "biases": biases})
    return in_maps


def kernel(x, Wf, bf, Wi, bi, Wh, bh):
    from concourse.bass_utils import run_bass_kernel_spmd

    in_maps = _make_in_maps(x, Wf, bf, Wi, bi, Wh, bh)
    nc = _get_nc()
    res = run_bass_kernel_spmd(nc, in_maps, list(range(NCORES)))

    out = np.empty((B, T + 1, H), dtype=np.float32)
    out[:, 0, :] = np.float32(0.5)
    for c in range(NCORES):
        out[c, 1:, :] = res.results[c]["out"].T
    return out



# revision 2
# speedup vs baseline: 1.3820x; 1.3820x over previous
"""MinLSTM fused kernel for TRN2 (8 NeuronCores, batch-parallel), bf16.

Math (equivalent to the reference's log-space form):
    zf = x@Wf+bf ; zi = x@Wi+bi ; zh = x@Wh+bh
    Ef = exp(-zf)               # 1/sigmoid(zf) = 1 + Ef
    si = sigmoid(zi) ; sh = sigmoid(zh)
    g  = max(zh + bh + 0.5, sh)
    p  = (1 + Ef) * si          # = si/sf
    S  = 0.5 + cumsum(p*g, axis=time)
    out[:, 0, :]  = 0.5
    out[:, t+1, :] = S[t] / (1 + p[t])

Per core (one batch element): bf16 GEMMs z^T [512h, 4096t] (stationary =
W chunk, moving = x^T chunk, 4 rotating PSUM slots of [128,1024]).
Elementwise in [H-partition, T-free]: ACT does Exp/Sigmoid/Reciprocal in
table-batched phases per h-chunk pair (exp -> sig -> recip); DVE does the
g/p STTs and the fp32-accum scan (bf16 out); Pool does the u and o
tensor muls (bf16). Output written bf16, host transposes + upconverts.
"""
import numpy as np

_CACHE = {}

B, T, D, H = 8, 4096, 512, 512
NCORES = 8
N_HC = H // 128       # 4 h-chunks
N_D = D // 128        # 4 contraction chunks
N_TC = 4              # 1024-wide T chunks per h for GEMM+ACT
TCW = 1024
HALF = 2048           # p/u/scan/r2/o granularity
XW_COLS = 3 * H + T   # 5632


def _install_tilefix():
    """This walrus build accepts only ONE sync wait per hardware instruction;
    Tile can emit several. Spill extras onto injected single-wait drains."""
    import concourse.tile as tile
    from concourse import mybir
    from concourse.vector_clock import ScopedClock

    if getattr(tile.TileContext, "_minlstm_patched", False):
        return
    orig_lower = tile.TileContext._lower_ordered_insts

    def _spill_waits(self, ordered):
        nc = self.nc
        for bb_name, insts in ordered.items():
            out = []
            for inst in insts:
                si = inst.sync_info
                if si is not None and len(si.on_wait) > 1 and inst.engine is not None:
                    waits = list(si.on_wait)
                    for w in waits[:-1]:
                        d = mybir.InstDrain(
                            name=nc.get_next_instruction_name(),
                            ins=[], outs=[], bass_is_fusable=False,
                            sync_info=mybir.SyncInfo(on_wait=[w], on_update=[]),
                        )
                        d.engine = inst.engine
                        out.append(d)
                    si.on_wait = [waits[-1]]
                out.append(inst)
            insts[:] = out
        return ordered

    def _patched_lower(self, ordered):
        return orig_lower(self, _spill_waits(self, ordered))

    def _split_drain_and_barrier(self, tick_clock, wait_clock):
        drain_inst = self.nc.sync.drain()
        wait_clock.add_sem_waits(
            drain_inst.ins, ScopedClock({None: tick_clock.global_clock})
        )
        si = drain_inst.ins.sync_info
        if si is not None and len(si.on_wait) > 1:
            waits = list(si.on_wait)
            si.on_wait = [waits[0]]
            for w in waits[1:]:
                extra = self.nc.sync.drain()
                esi = extra.ins.sync_info
                if esi is None:
                    extra.ins.sync_info = mybir.SyncInfo(on_wait=[w], on_update=[])
                else:
                    esi.on_wait = [w]
        self.nc.all_engine_barrier()
        assert self.sems is not None
        popped = self.nc._tile_sem_poison_stack.pop()
        assert popped is self._sem_poison
        self.nc.clear_and_free_semaphores(list(self.sems.allocated().values()))
        self.nc.all_engine_barrier()

    tile.TileContext._lower_ordered_insts = _patched_lower
    tile.TileContext._drain_and_barrier = _split_drain_and_barrier
    tile.TileContext._minlstm_patched = True


def _build():
    import concourse.bass as bass
    import concourse.tile as tile
    from concourse import mybir
    from concourse.tile_rust import add_dep_helper

    _install_tilefix()

    f32 = mybir.dt.float32
    bf16 = mybir.dt.bfloat16
    AF = mybir.ActivationFunctionType
    ALU = mybir.AluOpType

    nc = bass.Bass("TRN2", target_bir_lowering=False, debug=False,
                   num_devices=NCORES)

    xw_d = nc.dram_tensor("xw", [D, XW_COLS], bf16, kind="ExternalInput").ap()
    bias_d = nc.dram_tensor("biases", [128, 16], f32, kind="ExternalInput").ap()
    out_d = nc.dram_tensor("out", [H, T], bf16, kind="ExternalOutput").ap()

    prev_act = [None]

    def act_raw(out, in_, func, bias=0.0, scale=1.0):
        eng = nc.scalar
        inputs = [eng.lower_ap(in_)]
        for arg in (bias, scale, 0.0):
            if isinstance(arg, bass.AP):
                inputs.append(eng.lower_ap(arg))
            else:
                inputs.append(
                    mybir.ImmediateValue(dtype=f32, value=float(arg))
                )
        i = eng.add_instruction(
            mybir.InstActivation(
                name=nc.get_next_instruction_name(),
                func=func, ins=inputs, outs=[eng.lower_ap(out)],
            )
        )
        if prev_act[0] is not None:
            add_dep_helper(i.ins, prev_act[0].ins, sync=False,
                           reason="ACT table-set order")
        prev_act[0] = i
        return i

    with tile.TileContext(nc) as tc:
        with (
            tc.tile_pool(name="xwp", bufs=1) as xwp,
            tc.tile_pool(name="cons", bufs=1) as cons,
            tc.tile_pool(name="ps", bufs=4, space="PSUM") as ps,
            tc.tile_pool(name="grid", bufs=2) as grid,
            tc.tile_pool(name="shp", bufs=2) as shp,
            tc.tile_pool(name="pp", bufs=4) as pp,
            tc.tile_pool(name="up", bufs=2) as up,
            tc.tile_pool(name="Sp", bufs=2) as Sp,
            tc.tile_pool(name="rp", bufs=2) as rp,
            tc.tile_pool(name="op", bufs=4) as op,
        ):
            xw = [
                xwp.tile([128, XW_COLS], bf16, tag=f"xw{d}", name=f"xw{d}")
                for d in range(N_D)
            ]
            # W columns first, then x^T in 1024-wide chunks so the first
            # GEMMs can start early.
            for d in range(N_D):
                nc.sync.dma_start(
                    xw[d][:, 0:3 * H],
                    xw_d[128 * d:128 * (d + 1), 0:3 * H],
                )
            for tcol in range(N_TC):
                c0 = 3 * H + TCW * tcol
                for d in range(N_D):
                    nc.sync.dma_start(
                        xw[d][:, c0:c0 + TCW],
                        xw_d[128 * d:128 * (d + 1), c0:c0 + TCW],
                    )
            bt = cons.tile([128, 16], f32, tag="bt")
            nc.sync.dma_start(bt[:], bias_d[:])
            zero1 = cons.tile([128, 8], f32, tag="zero1")
            nc.vector.memset(zero1[:], 0.0)
            zb = zero1[:, 0:1].broadcast_to([128, HALF])

            def wchunk(gate, d, h):
                c = 512 * gate + 128 * h
                return xw[d][:, c:c + 128]

            def gemm(gate, h, tcol, name):
                z = ps.tile([128, TCW], f32, tag="z", name=name)
                for half in range(2):
                    sl = slice(512 * half, 512 * (half + 1))
                    t0 = 3 * H + TCW * tcol + 512 * half
                    for d in range(N_D):
                        nc.tensor.matmul(
                            z[:, sl], wchunk(gate, d, h),
                            xw[d][:, t0:t0 + 512],
                            start=(d == 0), stop=(d == N_D - 1),
                        )
                return z

            for pair in range(2):
                hs = (2 * pair, 2 * pair + 1)
                Ef = {}
                si = {}
                g = {}
                for h in hs:
                    Ef[h] = grid.tile([128, T], bf16, tag="Ef", name=f"Ef{h}")
                    si[h] = grid.tile([128, T], bf16, tag="si", name=f"si{h}")
                    g[h] = grid.tile([128, T], bf16, tag="g", name=f"g{h}")

                # ---- EXP phase: zf GEMMs -> Ef = exp(-zf) ----
                for h in hs:
                    nbf_ap = bt[:, h:h + 1]            # -bf
                    for tcol in range(N_TC):
                        z = gemm(0, h, tcol, f"zf{h}_{tcol}")
                        act_raw(Ef[h][:, TCW * tcol:TCW * (tcol + 1)], z[:],
                                AF.Exp, bias=nbf_ap, scale=-1.0)

                # ---- SIG phase: zi -> si ; zh -> sh, g ----
                for h in hs:
                    bi_ap = bt[:, 4 + h:5 + h]
                    bg_ap = bt[:, 8 + h:9 + h]         # bh + 0.5
                    bh_ap = bt[:, 12 + h:13 + h]
                    for tcol in range(N_TC):
                        zi = gemm(1, h, tcol, f"zi{h}_{tcol}")
                        act_raw(si[h][:, TCW * tcol:TCW * (tcol + 1)], zi[:],
                                AF.Sigmoid, bias=bi_ap)
                        zh = gemm(2, h, tcol, f"zh{h}_{tcol}")
                        sh = shp.tile([128, TCW], bf16, tag="sh", name=f"sh{h}_{tcol}")
                        act_raw(sh[:], zh[:], AF.Sigmoid, bias=bh_ap)
                        nc.vector.scalar_tensor_tensor(
                            out=g[h][:, TCW * tcol:TCW * (tcol + 1)],
                            in0=zh[:], scalar=bg_ap, in1=sh[:],
                            op0=ALU.add, op1=ALU.max,
                        )

                # ---- DVE p-STT, Pool u, DVE scan ----
                pt = {}
                St = {}
                for h in hs:
                    pt[h] = []
                    St[h] = []
                    S_prev = None
                    for half in range(2):
                        sl = slice(HALF * half, HALF * (half + 1))
                        p = pp.tile([128, HALF], bf16, tag="p",
                                    name=f"p{h}_{half}")
                        nc.vector.scalar_tensor_tensor(
                            out=p[:], in0=Ef[h][:, sl], scalar=1.0,
                            in1=si[h][:, sl], op0=ALU.add, op1=ALU.mult,
                        )
                        u = up.tile([128, HALF], bf16, tag="u",
                                    name=f"u{h}_{half}")
                        nc.gpsimd.tensor_tensor(out=u[:], in0=p[:],
                                                in1=g[h][:, sl], op=ALU.mult)
                        S = Sp.tile([128, HALF], bf16, tag="S",
                                    name=f"S{h}_{half}")
                        init = 0.5 if half == 0 else S_prev[:, HALF - 1:HALF]
                        nc.vector.tensor_tensor_scan(
                            S[:], zb, u[:], init, ALU.add, ALU.add
                        )
                        S_prev = S
                        pt[h].append(p)
                        St[h].append(S)

                # ---- RECIP phase: r2 = 1/(1+p); o = S*r2; DMA ----
                for h in hs:
                    for half in range(2):
                        r2 = rp.tile([128, HALF], bf16, tag="r2",
                                     name=f"r2_{h}_{half}")
                        act_raw(r2[:], pt[h][half][:], AF.Reciprocal, bias=1.0)
                        o = op.tile([128, HALF], bf16, tag="o",
                                    name=f"o{h}_{half}")
                        nc.gpsimd.tensor_tensor(out=o[:], in0=St[h][half][:],
                                                in1=r2[:], op=ALU.mult)
                        nc.sync.dma_start(
                            out_d[128 * h:128 * (h + 1),
                                  HALF * half:HALF * (half + 1)],
                            o[:],
                        )
    return nc


def _get_nc():
    if "nc" not in _CACHE:
        _CACHE["nc"] = _build()
    return _CACHE["nc"]


def _make_in_maps(x, Wf, bf, Wi, bi, Wh, bh):
    import ml_dtypes
    bft = ml_dtypes.bfloat16

    x = np.asarray(x, dtype=np.float32)
    W_all = np.concatenate(
        [np.asarray(Wf), np.asarray(Wi), np.asarray(Wh)], axis=1
    ).astype(bft)

    bf32 = np.asarray(bf, dtype=np.float32)
    bi32 = np.asarray(bi, dtype=np.float32)
    bh32 = np.asarray(bh, dtype=np.float32)
    biases = np.zeros((128, 16), dtype=np.float32)
    biases[:, 0:4] = (-bf32).reshape(N_HC, 128).T
    biases[:, 4:8] = bi32.reshape(N_HC, 128).T
    biases[:, 8:12] = (bh32 + np.float32(0.5)).reshape(N_HC, 128).T
    biases[:, 12:16] = bh32.reshape(N_HC, 128).T

    in_maps = []
    for c in range(NCORES):
        xT = np.ascontiguousarray(x[c].T).astype(bft)
        xw = np.concatenate([W_all, xT], axis=1)
        in_maps.append({"xw": xw, "biases": biases})
    return in_maps


def kernel(x, Wf, bf, Wi, bi, Wh, bh):
    from concourse.bass_utils import run_bass_kernel_spmd

    in_maps = _make_in_maps(x, Wf, bf, Wi, bi, Wh, bh)
    nc = _get_nc()
    res = run_bass_kernel_spmd(nc, in_maps, list(range(NCORES)))

    out = np.empty((B, T + 1, H), dtype=np.float32)
    out[:, 0, :] = np.float32(0.5)
    for c in range(NCORES):
        out[c, 1:, :] = np.asarray(res.results[c]["out"]).astype(np.float32).T
    return out
